# revision 51
# baseline (speedup 1.0000x reference)
"""Trainium2 Bass kernel for BasisFunction1D (piecewise-linear basis / histogram binning).

Math:
  out[o, b] = sum_i (1-d)*P[g, o, i] + d*P[g+1, o, i],
  g = bucket of x[i,b] on a Laplace-CDF grid, d = in-bucket linear position.

Key identity: with u = g + d the per-dim function is piecewise-linear in u with
knots at the integers, so using hat(v) = relu(v+1) - 2 relu(v) + relu(v-1):

  sum_g P_g * hat(u - g)  =  P_0 * (u+1)  +  sum_m Q_m * relu(u - m)

with Q_m = P_{m+1} - 2 P_m + P_{m-1} (second difference along g, built on host).
This needs ONE fused elementwise op per grid cell instead of the two-pass
|.| / min hat construction, and no per-input-dim PE broadcast at all.

fp32r matmul inputs are rounded to a ~10-bit mantissa, so raw relu(u-m) values
(up to 128) lose too much precision.  Two-level fix keeping every rhs in [0,8]:
with c~_q = min(u - 8q, 8)  (16 coarse tensors, one tensor_scalar each),

  relu(u - 8q - j) = max(c~_q - j, 0) + relu(u - 8(q+1))
  relu(u - 8k)     = sum_{k'>=k} max(c~_k', 0) + relu(u - 128)
  u                = sum_k max(c~_k, 0) + relu(u - 128) - relu(-u)

Folding the telescoped coarse sums into the host-side lhs table:

  out = C*1 + sum_{q,j} L_{q,j} * max(c~_q - j, 0)
        [+ OV * relu(u - 128) + RL * relu(-u)]   (zero on the closed-form path)

  L_{q,j} = Q_{8q+j} + (j==0) * (S'_q + P_0),  S'_q = sum_{q'<q} sum_j Q_{8q'+j}
  C = P_0,  OV = sum_m Q_m + P_0,  RL = P_0 - P_1

Every fine tile is ONE fused tensor_scalar / activation instruction, all rhs
values lie in [0,8] (exactly representable saturations; only the active
block's fractional value rounds), and all matmuls run fp32r at 1 cycle/row.

Device algorithm (per core, batch shard of 1024):
  1. Compute u[i,b] = g + d in closed form (the borders array is the
     inverse-Laplace-CDF grid; verified host-side, with an exact host-table
     fallback otherwise).
  2. Per q: c~_q on DVE, then 8 fine tiles split DVE/ACT by engine speed
     (gpsimd tensor_scalar measured ~12us/tile on HW - never used).
  3. out accumulates in PSUM over the matching fp32r matmuls against the
     host-side table [L_0..L_127, OV, RL]; column halves accumulate in two
     separate PSUM bank regions; the constant term C is added as a
     per-partition bias during PSUM->SBUF copy-out.
"""

import math

import numpy as np

I_DIM = 128
O_DIM = 128
G = 128
B_FULL = 8192
N_CORES = 8
BS = B_FULL // N_CORES
NBLK = 130  # L_0..L_127, OV, RL (constant term C folded into copy-out bias)

_NC_CACHE = {}


def _ref_grid_f64():
    def inv(u):
        return math.log(2.0 * u) if u <= 0.5 else -math.log(2.0 * (1.0 - u))

    cs = 1.0 / G
    b = [inv(i * cs) for i in range(1, G)]
    left = b[0] - (b[1] - b[0])
    right = b[-1] + (b[-1] - b[-2])
    return np.array([left] + b + [right], dtype=np.float64)


def _grid_matches(borders, inv_len):
    ref = _ref_grid_f64()
    ref32 = ref.astype(np.float32)
    il_ref = (1.0 / (ref32[1:].astype(np.float64) - ref32[:-1].astype(np.float64))).astype(
        np.float32
    )
    return np.allclose(borders, ref32, rtol=1e-5, atol=1e-5) and np.allclose(
        inv_len, il_ref, rtol=1e-4, atol=1e-4
    )


def _build_nc(host_tables: bool, reps: int = 1):
    from contextlib import ExitStack

    import concourse.bacc as bacc
    import concourse.mybir as mybir
    import concourse.tile as tile

    dt = mybir.dt
    f32 = dt.float32
    f32r = dt.float32r
    AF = mybir.ActivationFunctionType
    OP = mybir.AluOpType

    nc = bacc.Bacc("TRN2", target_bir_lowering=False, debug=False)

    x_d = nc.dram_tensor("x", [I_DIM, BS], f32, kind="ExternalInput")
    # [i, blk*o]: blk=f (0..127) is L_f^T; blk=128 is OV^T; blk=129 is RL^T
    qt_d = nc.dram_tensor("qt", [I_DIM, NBLK * O_DIM], f32r, kind="ExternalInput")
    # bias table: col m (0..127) = -m (ACT relu biases); col 128 = -64;
    # col 129 = -63.5; col 130 = C[o] (constant-term row sums, o-indexed)
    nb_d = nc.dram_tensor("nb", [128, 131], f32, kind="ExternalInput")
    if host_tables:
        hg_d = nc.dram_tensor("hg", [I_DIM, BS], f32, kind="ExternalInput")
        hb_d = nc.dram_tensor("hbor", [I_DIM, BS], f32, kind="ExternalInput")
        hi_d = nc.dram_tensor("hil", [I_DIM, BS], f32, kind="ExternalInput")
    out_d = nc.dram_tensor("out", [O_DIM, BS], f32, kind="ExternalOutput")

    # single phase per loop iteration: engines execute their instruction
    # queues in order, so a second phase's head just queues behind the first
    # phase's stream (measured ~3x slower per body when tried)
    n_ph = 1
    assert reps % n_ph == 0
    HB = BS // 2
    HALF = BS // 2

    with tile.TileContext(nc) as tc, ExitStack() as ctx:
        if reps > n_ph - 1 and reps // n_ph > 1:
            loop_cm = tc.For_i(
                0,
                reps // n_ph,
                1,
                hint_engines=(
                    mybir.EngineType.PE,
                    mybir.EngineType.Activation,
                    mybir.EngineType.DVE,
                ),
            )
            ctx.enter_context(loop_cm)
        pers = ctx.enter_context(tc.tile_pool(name="pers", bufs=1))
        scr = ctx.enter_context(tc.tile_pool(name="scr", bufs=1))
        rpools = [
            ctx.enter_context(tc.tile_pool(name=f"rpool{p}", bufs=6))
            for p in range(n_ph)
        ]
        cpools = [
            ctx.enter_context(tc.tile_pool(name=f"cpool{p}", bufs=4))
            for p in range(n_ph)
        ]
        opsum = ctx.enter_context(tc.tile_pool(name="opsum", bufs=1, space="PSUM"))

        nb_sb = pers.tile([128, 131], f32, tag="nb", name="nb")
        nc.sync.dma_start(nb_sb[:], nb_d.ap())
        qt_sb = pers.tile([I_DIM, NBLK * O_DIM], f32r, tag="qt", name="qt")
        out_sb = pers.tile([O_DIM, BS], f32, tag="osb", name="osb")

        NCH = 8
        per = (NBLK + NCH - 1) // NCH

        def emit_body(p):
            x_sb = pers.tile([I_DIM, BS], f32, tag=f"x{p}", name=f"x{p}")
            nc.sync.dma_start(x_sb[:], x_d.ap())

            # Q table in chunks so early matmuls don't wait on the full load
            for c in range(NCH):
                lo = c * per * O_DIM
                hi = min(NBLK, (c + 1) * per) * O_DIM
                nc.sync.dma_start(qt_sb[:, lo:hi], qt_d.ap()[:, lo:hi])

            def sct(tag, dtype=f32):
                return scr.tile([I_DIM, BS // 4], dtype, tag=tag, name=tag)

            u = pers.tile([I_DIM, BS], f32, tag=f"u{p}", name=f"u{p}")

            if not host_tables:
                # ---- closed-form u = g + (x - borders[g]) * inv_len[g] ----
                # 4 column chunks: the ~25-stage chain pipelines across
                # chunks, so the full u tensor is ready sooner and the PE's
                # initial idle gap shrinks
                gf = pers.tile([I_DIM, BS], f32, tag=f"gf{p}", name=f"gf{p}")
                NCK = 4
                CW = BS // NCK
                for h in range(NCK):
                    cs = slice(h * CW, (h + 1) * CW)
                    ax = sct(f"T0{h}")
                    nc.scalar.activation(ax[:], x_sb[:, cs], AF.Abs)
                    e = sct(f"T1{h}")
                    nc.scalar.activation(e[:], ax[:], AF.Exp, scale=-1.0)
                    s = sct(f"T2{h}")
                    nc.scalar.activation(s[:], x_sb[:, cs], AF.Sign)
                    se = sct(f"T0{h}")
                    nc.gpsimd.tensor_mul(se[:], s[:], e[:])
                    t1 = sct(f"T1{h}")
                    nc.vector.tensor_sub(t1[:], s[:], se[:])
                    # y = cdf * 128 = 64 + 64*s*(1-e)
                    y = sct(f"T0{h}")
                    nc.vector.tensor_scalar(y[:], t1[:], 64.0, 64.0, OP.mult, OP.add)
                    gi = scr.tile([I_DIM, BS // 4], dt.int32, tag=f"T3{h}", name=f"T3{h}")
                    nc.vector.tensor_copy(gi[:], y[:])
                    g0 = sct(f"T1{h}")
                    nc.vector.tensor_copy(g0[:], gi[:])
                    # robust floor regardless of the converter rounding mode;
                    # no clamp needed: y = 64 + 64*s*(1-e) lies strictly in
                    # (0.3, 127.7) for |x| < 16, so floor(y) is in [0,127]
                    cg = sct(f"T4{h}")
                    nc.vector.tensor_tensor(cg[:], g0[:], y[:], op=OP.is_gt)
                    nc.vector.tensor_sub(gf[:, cs], g0[:], cg[:])
                    # borders[g] = sign * ln(m1/64), m1 = max(min(g, 128-g), 0.5)
                    a1 = sct(f"T1{h}")
                    nc.scalar.activation(a1[:], gf[:, cs], AF.Abs, bias=nb_sb[:, 128:129])
                    a1c = sct(f"T3{h}")
                    nc.vector.tensor_scalar(a1c[:], a1[:], 63.5, None, OP.min)
                    L = sct(f"T1{h}")
                    nc.scalar.activation(L[:], a1c[:], AF.Ln, scale=-1.0 / 64.0, bias=1.0)
                    sL = sct(f"T3{h}")
                    nc.gpsimd.tensor_mul(sL[:], s[:], L[:])
                    xb = sct(f"T2{h}")  # x - borders[g]  (= x + s*L)
                    nc.gpsimd.tensor_add(xb[:], x_sb[:, cs], sL[:])
                    # inv_len[g] = 1/ln(1 + 1/m2), m2 = max(min(g, 127-g), 1)
                    a2 = sct(f"T0{h}")
                    nc.scalar.activation(a2[:], gf[:, cs], AF.Abs, bias=nb_sb[:, 129:130])
                    nm2 = sct(f"T1{h}")  # -m2
                    nc.vector.tensor_scalar(nm2[:], a2[:], 63.5, -1.0, OP.subtract, OP.min)
                    rm = sct(f"T0{h}")  # -1/m2
                    nc.vector.reciprocal_approx_fast(rm[:], nm2[:])
                    q = sct(f"T1{h}")  # ln(1 + 1/m2)
                    nc.scalar.activation(q[:], rm[:], AF.Ln, scale=-1.0, bias=1.0)
                    il = sct(f"T0{h}")
                    nc.vector.reciprocal_approx_fast(il[:], q[:])
                    d_ = sct(f"T1{h}")
                    nc.vector.tensor_mul(d_[:], xb[:], il[:])
                    nc.vector.tensor_add(u[:, cs], gf[:, cs], d_[:])
            else:
                hg_sb = pers.tile([I_DIM, BS], f32, tag=f"hg{p}", name=f"hg{p}")
                nc.sync.dma_start(hg_sb[:], hg_d.ap())
                hb_sb = scr.tile([I_DIM, BS], f32, tag="H0", name="H0")
                nc.sync.dma_start(hb_sb[:], hb_d.ap())
                hi_sb = scr.tile([I_DIM, BS], f32, tag="H1", name="H1")
                nc.sync.dma_start(hi_sb[:], hi_d.ap())
                xb = scr.tile([I_DIM, BS], f32, tag="H2", name="H2")
                nc.vector.tensor_sub(xb[:], x_sb[:], hb_sb[:])
                d_ = scr.tile([I_DIM, BS], f32, tag="H0", name="H0")
                nc.vector.tensor_mul(d_[:], xb[:], hi_sb[:])
                nc.vector.tensor_add(u[:], hg_sb[:], d_[:])

            # ---- main loop: rhs tile then its two accumulate matmuls ----
            acc0 = opsum.tile([O_DIM, HALF], f32, tag=f"acc0{p}", name=f"acc0{p}")
            acc1 = opsum.tile([O_DIM, HALF], f32, tag=f"acc1{p}", name=f"acc1{p}")
            rpool, cpool = rpools[p], cpools[p]

            def mm_pair(blk, t):
                # column halves go to separate PSUM regions
                first = blk == 0
                last = blk == NBLK - 1
                lhsT = qt_sb[:, blk * O_DIM : (blk + 1) * O_DIM]
                nc.tensor.matmul(
                    acc0[:, :], lhsT, t[:, 0:HALF],
                    start=first, stop=last, skip_group_check=True,
                )
                nc.tensor.matmul(
                    acc1[:, :], lhsT, t[:, HALF:BS],
                    start=first, stop=last, skip_group_check=True,
                )

            # deterministic weighted assignment of the 128 fine tiles
            # (gpsimd tensor_scalar measured ~12us/tile on HW - never use it)
            share = {"D": 76.0 / 128, "A": 52.0 / 128}
            used = {"D": 0, "A": 0}
            for q in range(16):
                ct = cpool.tile([I_DIM, BS], f32, tag="c", name="c")
                nc.vector.tensor_scalar(ct[:], u[:], 8.0 * q, 8.0, OP.subtract, OP.min)
                for j in range(8):
                    f = 8 * q + j
                    eng = max("DA", key=lambda e: share[e] * (f + 1) - used[e])
                    used[eng] += 1
                    t = rpool.tile([I_DIM, BS], f32r, tag="r", name="r")
                    if eng == "A":
                        nc.scalar.activation(t[:], ct[:], AF.Relu, bias=nb_sb[:, j : j + 1])
                    else:
                        nc.vector.tensor_scalar(t[:], ct[:], float(j), 0.0, OP.subtract, OP.max)
                    mm_pair(f, t)
            # u < 0 and u > 128 occur even on the closed-form path (the two
            # extrapolation buckets reach past the borders array)
            ov = rpool.tile([I_DIM, BS], f32r, tag="r", name="r")
            nc.vector.tensor_scalar(ov[:], u[:], 128.0, 0.0, OP.subtract, OP.max)
            mm_pair(128, ov)
            rl = rpool.tile([I_DIM, BS], f32r, tag="r", name="r")
            nc.vector.tensor_scalar(rl[:], u[:], -1.0, 0.0, OP.mult, OP.max)
            mm_pair(129, rl)

            # constant term C (folded ones-block) is added during copy-out
            nc.vector.tensor_scalar(out_sb[:, 0:HALF], acc0[:, :], nb_sb[:, 130:131], None, OP.add)
            nc.scalar.activation(out_sb[:, HALF:BS], acc1[:, :], AF.Identity, bias=nb_sb[:, 130:131])
            nc.sync.dma_start(out_d.ap(), out_sb[:])

        for p in range(n_ph):
            emit_body(p)

    return nc


def _get_nc(host_tables: bool, reps: int = 1):
    key = (bool(host_tables), reps)
    if key not in _NC_CACHE:
        _NC_CACHE[key] = _build_nc(key[0], reps)
    return _NC_CACHE[key]


def _host_inputs(x, func_parameter, borders, inverse_chunk_lengths):
    x = np.ascontiguousarray(np.asarray(x, dtype=np.float32))
    P = np.asarray(func_parameter, dtype=np.float32)
    borders = np.asarray(borders, dtype=np.float32)
    inv_len = np.asarray(inverse_chunk_lengths, dtype=np.float32)

    host_tables = not _grid_matches(borders, inv_len)

    # blocks in use order: L_0..L_127, OV, RL   (each [O, I])
    Pd = P.astype(np.float64)
    Qm = Pd[1 : G + 1] - 2.0 * Pd[0:G]
    Qm[1:] += Pd[0 : G - 1]  # Q_m = P_{m+1} - 2 P_m + P_{m-1}
    R = Qm.reshape(16, 8, O_DIM, I_DIM).sum(axis=1)  # [16, O, I]
    Sp = np.cumsum(R, axis=0) - R  # S'_q = sum_{q'<q} R_{q'}
    L = Qm.reshape(16, 8, O_DIM, I_DIM).copy()
    L[:, 0] += Sp + Pd[0]
    blocks = np.concatenate(
        [
            L.reshape(G, O_DIM, I_DIM),
            (Qm.sum(axis=0) + Pd[0])[None],  # OV
            (Pd[0] - Pd[1])[None],  # RL
        ],
        axis=0,
    ).astype(np.float32)  # [130, O, I]
    qt = np.ascontiguousarray(blocks.transpose(2, 0, 1).reshape(I_DIM, NBLK * O_DIM))

    nb = np.zeros((128, 131), dtype=np.float32)
    nb[:, 0:128] = -np.arange(128, dtype=np.float32)[None, :]
    nb[:, 128] = -64.0
    nb[:, 129] = -63.5
    nb[:, 130] = Pd[0].sum(axis=1).astype(np.float32)  # C[o] = sum_i P_0[o,i]

    in_maps = []
    for c in range(N_CORES):
        xs = np.ascontiguousarray(x[:, c * BS : (c + 1) * BS])
        m = {"x": xs, "qt": qt, "nb": nb}
        if host_tables:
            # exact fallback: bucketize host-side with the provided tables
            exp_na = np.exp(-np.abs(xs))
            cdf = np.where(xs > 0, 1.0 - 0.5 * exp_na, 0.5 * exp_na).astype(np.float32)
            idx = np.clip((cdf * G).astype(np.int32), 0, G - 1)
            m["hg"] = idx.astype(np.float32)
            m["hbor"] = borders[idx].astype(np.float32)
            m["hil"] = inv_len[idx].astype(np.float32)
        in_maps.append(m)
    return in_maps, host_tables


_RUNNER_CACHE = {}


def _get_runner(host_tables, reps: int = 1):
    """Cached jitted 8-core runner (mirrors bass2jax.run_bass_via_pjrt multi-core path)."""
    key = (bool(host_tables), reps)
    if key in _RUNNER_CACHE:
        return _RUNNER_CACHE[key]

    import jax
    from jax.sharding import Mesh, PartitionSpec
    from jax.experimental.shard_map import shard_map
    import concourse.mybir as mybir
    from concourse.bass2jax import (
        _bass_exec_p,
        install_neuronx_cc_hook,
        partition_id_tensor,
    )

    install_neuronx_cc_hook()
    nc = _get_nc(host_tables, reps)
    if not nc.is_finalized():
        nc.finalize()
    assert nc.dbg_addr is None
    partition_name = nc.partition_id_tensor.name if nc.partition_id_tensor else None

    in_names, out_names, out_avals, zero_outs = [], [], [], []
    for alloc in nc.m.functions[0].allocations:
        if not isinstance(alloc, mybir.MemoryLocationSet):
            continue
        name = alloc.memorylocations[0].name
        if alloc.kind == "ExternalInput":
            if name != partition_name:
                in_names.append(name)
        elif alloc.kind == "ExternalOutput":
            shape = tuple(alloc.tensor_shape)
            dtype = mybir.dt.np(alloc.dtype)
            out_names.append(name)
            out_avals.append(jax.core.ShapedArray(shape, dtype))
            zero_outs.append(np.zeros(shape, dtype))
    n_params = len(in_names)
    all_names = in_names + out_names
    if partition_name is not None:
        all_names = all_names + [partition_name]

    def _body(*args):
        operands = list(args)
        if partition_name is not None:
            operands.append(partition_id_tensor())
        outs = _bass_exec_p.bind(
            *operands,
            out_avals=tuple(out_avals),
            in_names=tuple(all_names),
            out_names=tuple(out_names),
            lowering_input_output_aliases=(),
            sim_require_finite=True,
            sim_require_nnan=True,
            nc=nc,
        )
        return tuple(outs)

    devices = jax.devices()[:N_CORES]
    mesh = Mesh(np.asarray(devices), ("core",))
    n_outs = len(out_names)
    sharded = jax.jit(
        shard_map(
            _body,
            mesh=mesh,
            in_specs=(PartitionSpec("core"),) * (n_params + n_outs),
            out_specs=(PartitionSpec("core"),) * n_outs,
            check_rep=False,
        ),
        keep_unused=True,
    )

    def run(in_maps):
        concat_in = [
            np.concatenate([np.asarray(m[name]) for m in in_maps], axis=0)
            for name in in_names
        ]
        concat_zero = [
            np.zeros((N_CORES * z.shape[0], *z.shape[1:]), z.dtype) for z in zero_outs
        ]
        out_arrs = sharded(*concat_in, *concat_zero)
        res = [
            {
                name: np.asarray(out_arrs[i]).reshape(N_CORES, *out_avals[i].shape)[c]
                for i, name in enumerate(out_names)
            }
            for c in range(N_CORES)
        ]
        return res, (sharded, concat_in, concat_zero)

    _RUNNER_CACHE[key] = run
    return run


def _run(in_maps, host_tables, trace=False):
    run = _get_runner(host_tables)
    results, _ = run(in_maps)
    out = np.concatenate([r["out"] for r in results], axis=1)
    return np.ascontiguousarray(out.astype(np.float32)), results


def bench(in_maps, host_tables, iters=30, reps=1):
    """Return (best_per_exec_seconds, times list) by timing repeated dispatches."""
    import time
    import jax

    run = _get_runner(host_tables, reps)
    _, (sharded, concat_in, concat_zero) = run(in_maps)
    # device-resident inputs to avoid re-transfer
    din = [jax.device_put(a) for a in concat_in]
    dzero = [jax.device_put(a) for a in concat_zero]
    jax.block_until_ready(sharded(*din, *dzero))
    times = []
    for _ in range(iters):
        t0 = time.perf_counter()
        jax.block_until_ready(sharded(*din, *dzero))
        times.append(time.perf_counter() - t0)
    return min(times), times


def bench_device(in_maps, host_tables, reps=256, iters=10):
    """Estimate true per-kernel device time from the marginal cost between two
    loop counts of the SAME kernel structure.  The two dispatches are timed
    interleaved so slow drift in the (dominant, ~100ms) axon dispatch
    overhead cancels instead of biasing the difference."""
    import time
    import jax

    lo = max(2, reps // 4)
    runs = {}
    for r in (lo, reps):
        run = _get_runner(host_tables, r)
        _, (sharded, concat_in, concat_zero) = run(in_maps)
        din = [jax.device_put(a) for a in concat_in]
        dzero = [jax.device_put(a) for a in concat_zero]
        jax.block_until_ready(sharded(*din, *dzero))
        runs[r] = (sharded, din, dzero)
    t = {lo: [], reps: []}
    for _ in range(max(iters, 20)):
        for r in (lo, reps):
            sharded, din, dzero = runs[r]
            t0 = time.perf_counter()
            jax.block_until_ready(sharded(*din, *dzero))
            t[r].append(time.perf_counter() - t0)
    # dispatch overhead drifts between rounds but is common-mode within a
    # round, so the median of same-round differences is a robust estimate of
    # the marginal per-body device time
    diffs = sorted(b - a for a, b in zip(t[lo], t[reps]))
    per = diffs[len(diffs) // 2] / (reps - lo)
    return per, min(t[lo]), min(t[reps])


def kernel(x, func_parameter, borders, inverse_chunk_lengths):
    in_maps, host_tables = _host_inputs(x, func_parameter, borders, inverse_chunk_lengths)
    out, _ = _run(in_maps, host_tables, trace=False)
    return out


def kernel_with_stats(x, func_parameter, borders, inverse_chunk_lengths, trace=True):
    """Returns (out, results) - test harness helper."""
    in_maps, host_tables = _host_inputs(x, func_parameter, borders, inverse_chunk_lengths)
    out, results = _run(in_maps, host_tables)
    return out, (in_maps, host_tables)


# revision 52
# speedup vs baseline: 1.2409x; 1.2409x over previous
"""Trainium2 Bass kernel for BasisFunction1D (piecewise-linear basis / histogram binning).

Math:
  out[o, b] = sum_i (1-d)*P[g, o, i] + d*P[g+1, o, i],
  g = bucket of x[i,b] on a Laplace-CDF grid, d = in-bucket linear position.

Key identity: with u = g + d the per-dim function is piecewise-linear in u with
knots at the integers, so using hat(v) = relu(v+1) - 2 relu(v) + relu(v-1):

  sum_g P_g * hat(u - g)  =  P_0 * (u+1)  +  sum_m Q_m * relu(u - m)

with Q_m = P_{m+1} - 2 P_m + P_{m-1} (second difference along g, built on host).
This needs ONE fused elementwise op per grid cell instead of the two-pass
|.| / min hat construction, and no per-input-dim PE broadcast at all.

fp32r matmul inputs are rounded to a ~10-bit mantissa, so raw relu(u-m) values
(up to 128) lose too much precision.  Two-level fix keeping every rhs in [0,8]:
with c~_q = min(u - 8q, 8)  (16 coarse tensors, one tensor_scalar each),

  relu(u - 8q - j) = max(c~_q - j, 0) + relu(u - 8(q+1))
  relu(u - 8k)     = sum_{k'>=k} max(c~_k', 0) + relu(u - 128)
  u                = sum_k max(c~_k, 0) + relu(u - 128) - relu(-u)

Folding the telescoped coarse sums into the host-side lhs table:

  out = C*1 + sum_{q,j} L_{q,j} * max(c~_q - j, 0)
        [+ OV * relu(u - 128) + RL * relu(-u)]   (zero on the closed-form path)

  L_{q,j} = Q_{8q+j} + (j==0) * (S'_q + P_0),  S'_q = sum_{q'<q} sum_j Q_{8q'+j}
  C = P_0,  OV = sum_m Q_m + P_0,  RL = P_0 - P_1

Every fine tile is ONE fused tensor_scalar / activation instruction, all rhs
values lie in [0,8] (exactly representable saturations; only the active
block's fractional value rounds), and all matmuls run fp32r at 1 cycle/row.

Device algorithm (per core, batch shard of 1024):
  1. Compute u[i,b] = g + d in closed form (the borders array is the
     inverse-Laplace-CDF grid; verified host-side, with an exact host-table
     fallback otherwise).
  2. Per q: c~_q on DVE, then 8 fine tiles split DVE/ACT by engine speed
     (gpsimd tensor_scalar measured ~12us/tile on HW - never used).
  3. out accumulates in PSUM over the matching fp32r matmuls against the
     host-side table [L_0..L_127, OV, RL]; column halves accumulate in two
     separate PSUM bank regions; the constant term C is added as a
     per-partition bias during PSUM->SBUF copy-out.
"""

import math

import numpy as np

I_DIM = 128
O_DIM = 128
G = 128
B_FULL = 8192
N_CORES = 8
BS = B_FULL // N_CORES
NBLK = 130  # L_0..L_127, OV, RL (constant term C folded into copy-out bias)

_NC_CACHE = {}


def _ref_grid_f64():
    def inv(u):
        return math.log(2.0 * u) if u <= 0.5 else -math.log(2.0 * (1.0 - u))

    cs = 1.0 / G
    b = [inv(i * cs) for i in range(1, G)]
    left = b[0] - (b[1] - b[0])
    right = b[-1] + (b[-1] - b[-2])
    return np.array([left] + b + [right], dtype=np.float64)


def _grid_matches(borders, inv_len):
    ref = _ref_grid_f64()
    ref32 = ref.astype(np.float32)
    il_ref = (1.0 / (ref32[1:].astype(np.float64) - ref32[:-1].astype(np.float64))).astype(
        np.float32
    )
    return np.allclose(borders, ref32, rtol=1e-5, atol=1e-5) and np.allclose(
        inv_len, il_ref, rtol=1e-4, atol=1e-4
    )


def _build_nc(host_tables: bool, reps: int = 1):
    from contextlib import ExitStack

    import concourse.bacc as bacc
    import concourse.mybir as mybir
    import concourse.tile as tile

    dt = mybir.dt
    f32 = dt.float32
    f32r = dt.float32r
    AF = mybir.ActivationFunctionType
    OP = mybir.AluOpType

    nc = bacc.Bacc("TRN2", target_bir_lowering=False, debug=False)

    x_d = nc.dram_tensor("x", [I_DIM, BS], f32, kind="ExternalInput")
    # [i, blk*o]: blk=f (0..127) is L_f^T; blk=128 is OV^T; blk=129 is RL^T
    qt_d = nc.dram_tensor("qt", [I_DIM, NBLK * O_DIM], f32r, kind="ExternalInput")
    # bias table: col m (0..127) = -m (ACT relu biases); col 128 = -64;
    # col 129 = -63.5; col 130 = C[o] (constant-term row sums, o-indexed)
    nb_d = nc.dram_tensor("nb", [128, 131], f32, kind="ExternalInput")
    if host_tables:
        hg_d = nc.dram_tensor("hg", [I_DIM, BS], f32, kind="ExternalInput")
        hb_d = nc.dram_tensor("hbor", [I_DIM, BS], f32, kind="ExternalInput")
        hi_d = nc.dram_tensor("hil", [I_DIM, BS], f32, kind="ExternalInput")
    out_d = nc.dram_tensor("out", [O_DIM, BS], f32, kind="ExternalOutput")

    # single phase per loop iteration: engines execute their instruction
    # queues in order, so a second phase's head just queues behind the first
    # phase's stream (measured ~3x slower per body when tried)
    n_ph = 1
    assert reps % n_ph == 0
    HB = BS // 2
    HALF = BS // 2

    with tile.TileContext(nc) as tc, ExitStack() as ctx:
        if reps > n_ph - 1 and reps // n_ph > 1:
            loop_cm = tc.For_i(
                0,
                reps // n_ph,
                1,
                hint_engines=(
                    mybir.EngineType.PE,
                    mybir.EngineType.Activation,
                    mybir.EngineType.DVE,
                ),
            )
            ctx.enter_context(loop_cm)
        pers = ctx.enter_context(tc.tile_pool(name="pers", bufs=1))
        scr = ctx.enter_context(tc.tile_pool(name="scr", bufs=1))
        rpools = [
            ctx.enter_context(tc.tile_pool(name=f"rpool{p}", bufs=6))
            for p in range(n_ph)
        ]
        cpools = [
            ctx.enter_context(tc.tile_pool(name=f"cpool{p}", bufs=4))
            for p in range(n_ph)
        ]
        opsum = ctx.enter_context(tc.tile_pool(name="opsum", bufs=1, space="PSUM"))

        nb_sb = pers.tile([128, 131], f32, tag="nb", name="nb")
        nc.sync.dma_start(nb_sb[:], nb_d.ap())
        qt_sb = pers.tile([I_DIM, NBLK * O_DIM], f32r, tag="qt", name="qt")
        out_sb = pers.tile([O_DIM, BS], f32, tag="osb", name="osb")

        NCH = 8
        per = (NBLK + NCH - 1) // NCH

        def emit_body(p):
            x_sb = pers.tile([I_DIM, BS], f32, tag=f"x{p}", name=f"x{p}")
            nc.sync.dma_start(x_sb[:], x_d.ap())

            # Q table in chunks so early matmuls don't wait on the full load
            for c in range(NCH):
                lo = c * per * O_DIM
                hi = min(NBLK, (c + 1) * per) * O_DIM
                nc.sync.dma_start(qt_sb[:, lo:hi], qt_d.ap()[:, lo:hi])

            def sct(tag, dtype=f32):
                return scr.tile([I_DIM, BS // 4], dtype, tag=tag, name=tag)

            u = pers.tile([I_DIM, BS], f32, tag=f"u{p}", name=f"u{p}")

            if not host_tables:
                # ---- closed-form u = g + (x - borders[g]) * inv_len[g] ----
                # 4 column chunks: the ~25-stage chain pipelines across
                # chunks, so the full u tensor is ready sooner and the PE's
                # initial idle gap shrinks
                gf = pers.tile([I_DIM, BS], f32, tag=f"gf{p}", name=f"gf{p}")
                NCK = 4
                CW = BS // NCK
                for h in range(NCK):
                    cs = slice(h * CW, (h + 1) * CW)
                    ax = sct(f"T0{h}")
                    nc.scalar.activation(ax[:], x_sb[:, cs], AF.Abs)
                    e = sct(f"T1{h}")
                    nc.scalar.activation(e[:], ax[:], AF.Exp, scale=-1.0)
                    s = sct(f"T2{h}")
                    nc.scalar.activation(s[:], x_sb[:, cs], AF.Sign)
                    se = sct(f"T0{h}")
                    nc.gpsimd.tensor_mul(se[:], s[:], e[:])
                    t1 = sct(f"T1{h}")
                    nc.vector.tensor_sub(t1[:], s[:], se[:])
                    # y = cdf * 128 = 64 + 64*s*(1-e)
                    y = sct(f"T0{h}")
                    nc.vector.tensor_scalar(y[:], t1[:], 64.0, 64.0, OP.mult, OP.add)
                    gi = scr.tile([I_DIM, BS // 4], dt.int32, tag=f"T3{h}", name=f"T3{h}")
                    nc.vector.tensor_copy(gi[:], y[:])
                    g0 = sct(f"T1{h}")
                    nc.vector.tensor_copy(g0[:], gi[:])
                    # robust floor regardless of the converter rounding mode;
                    # no clamp needed: y = 64 + 64*s*(1-e) lies strictly in
                    # (0.3, 127.7) for |x| < 16, so floor(y) is in [0,127]
                    cg = sct(f"T4{h}")
                    nc.vector.tensor_tensor(cg[:], g0[:], y[:], op=OP.is_gt)
                    nc.vector.tensor_sub(gf[:, cs], g0[:], cg[:])
                    # borders[g] = sign * ln(m1/64), m1 = max(min(g, 128-g), 0.5)
                    a1 = sct(f"T1{h}")
                    nc.scalar.activation(a1[:], gf[:, cs], AF.Abs, bias=nb_sb[:, 128:129])
                    a1c = sct(f"T3{h}")
                    nc.vector.tensor_scalar(a1c[:], a1[:], 63.5, None, OP.min)
                    L = sct(f"T1{h}")
                    nc.scalar.activation(L[:], a1c[:], AF.Ln, scale=-1.0 / 64.0, bias=1.0)
                    sL = sct(f"T3{h}")
                    nc.gpsimd.tensor_mul(sL[:], s[:], L[:])
                    xb = sct(f"T2{h}")  # x - borders[g]  (= x + s*L)
                    nc.gpsimd.tensor_add(xb[:], x_sb[:, cs], sL[:])
                    # inv_len[g] = 1/ln(1 + 1/m2), m2 = max(min(g, 127-g), 1)
                    a2 = sct(f"T0{h}")
                    nc.scalar.activation(a2[:], gf[:, cs], AF.Abs, bias=nb_sb[:, 129:130])
                    nm2 = sct(f"T1{h}")  # -m2
                    nc.vector.tensor_scalar(nm2[:], a2[:], 63.5, -1.0, OP.subtract, OP.min)
                    rm = sct(f"T0{h}")  # -1/m2
                    nc.vector.reciprocal_approx_fast(rm[:], nm2[:])
                    q = sct(f"T1{h}")  # ln(1 + 1/m2)
                    nc.scalar.activation(q[:], rm[:], AF.Ln, scale=-1.0, bias=1.0)
                    il = sct(f"T0{h}")
                    nc.vector.reciprocal_approx_fast(il[:], q[:])
                    d_ = sct(f"T1{h}")
                    nc.vector.tensor_mul(d_[:], xb[:], il[:])
                    nc.vector.tensor_add(u[:, cs], gf[:, cs], d_[:])
            else:
                hg_sb = pers.tile([I_DIM, BS], f32, tag=f"hg{p}", name=f"hg{p}")
                nc.sync.dma_start(hg_sb[:], hg_d.ap())
                hb_sb = scr.tile([I_DIM, BS], f32, tag="H0", name="H0")
                nc.sync.dma_start(hb_sb[:], hb_d.ap())
                hi_sb = scr.tile([I_DIM, BS], f32, tag="H1", name="H1")
                nc.sync.dma_start(hi_sb[:], hi_d.ap())
                xb = scr.tile([I_DIM, BS], f32, tag="H2", name="H2")
                nc.vector.tensor_sub(xb[:], x_sb[:], hb_sb[:])
                d_ = scr.tile([I_DIM, BS], f32, tag="H0", name="H0")
                nc.vector.tensor_mul(d_[:], xb[:], hi_sb[:])
                nc.vector.tensor_add(u[:], hg_sb[:], d_[:])

            # ---- main loop: rhs tile then its two accumulate matmuls ----
            acc0 = opsum.tile([O_DIM, HALF], f32, tag=f"acc0{p}", name=f"acc0{p}")
            acc1 = opsum.tile([O_DIM, HALF], f32, tag=f"acc1{p}", name=f"acc1{p}")
            rpool, cpool = rpools[p], cpools[p]

            def mm_pair(blk, t):
                # column halves go to separate PSUM regions
                first = blk == 0
                last = blk == NBLK - 1
                lhsT = qt_sb[:, blk * O_DIM : (blk + 1) * O_DIM]
                nc.tensor.matmul(
                    acc0[:, :], lhsT, t[:, 0:HALF],
                    start=first, stop=last, skip_group_check=True,
                )
                nc.tensor.matmul(
                    acc1[:, :], lhsT, t[:, HALF:BS],
                    start=first, stop=last, skip_group_check=True,
                )

            # deterministic weighted assignment of the 128 fine tiles
            # (gpsimd tensor_scalar measured ~12us/tile on HW - never use it)
            share = {"D": 76.0 / 128, "A": 52.0 / 128}
            used = {"D": 0, "A": 0}
            for q in range(16):
                ct = cpool.tile([I_DIM, BS], f32, tag="c", name="c")
                nc.vector.tensor_scalar(ct[:], u[:], 8.0 * q, 8.0, OP.subtract, OP.min)
                for j in range(8):
                    f = 8 * q + j
                    eng = max("DA", key=lambda e: share[e] * (f + 1) - used[e])
                    used[eng] += 1
                    t = rpool.tile([I_DIM, BS], f32r, tag="r", name="r")
                    if eng == "A":
                        nc.scalar.activation(t[:], ct[:], AF.Relu, bias=nb_sb[:, j : j + 1])
                    else:
                        nc.vector.tensor_scalar(t[:], ct[:], float(j), 0.0, OP.subtract, OP.max)
                    mm_pair(f, t)
            # u < 0 and u > 128 occur even on the closed-form path (the two
            # extrapolation buckets reach past the borders array)
            ov = rpool.tile([I_DIM, BS], f32r, tag="r", name="r")
            nc.vector.tensor_scalar(ov[:], u[:], 128.0, 0.0, OP.subtract, OP.max)
            mm_pair(128, ov)
            rl = rpool.tile([I_DIM, BS], f32r, tag="r", name="r")
            nc.vector.tensor_scalar(rl[:], u[:], -1.0, 0.0, OP.mult, OP.max)
            mm_pair(129, rl)

            # constant term C (folded ones-block) is added during copy-out
            nc.vector.tensor_scalar(out_sb[:, 0:HALF], acc0[:, :], nb_sb[:, 130:131], None, OP.add)
            nc.scalar.activation(out_sb[:, HALF:BS], acc1[:, :], AF.Identity, bias=nb_sb[:, 130:131])
            nc.sync.dma_start(out_d.ap(), out_sb[:])

        for p in range(n_ph):
            emit_body(p)

    return nc


def _get_nc(host_tables: bool, reps: int = 1):
    key = (bool(host_tables), reps)
    if key not in _NC_CACHE:
        _NC_CACHE[key] = _build_nc(key[0], reps)
    return _NC_CACHE[key]


def _host_inputs(x, func_parameter, borders, inverse_chunk_lengths):
    x = np.ascontiguousarray(np.asarray(x, dtype=np.float32))
    P = np.asarray(func_parameter, dtype=np.float32)
    borders = np.asarray(borders, dtype=np.float32)
    inv_len = np.asarray(inverse_chunk_lengths, dtype=np.float32)

    host_tables = not _grid_matches(borders, inv_len)

    # blocks in use order: L_0..L_127, OV, RL   (each [O, I])
    Pd = P.astype(np.float64)
    Qm = Pd[1 : G + 1] - 2.0 * Pd[0:G]
    Qm[1:] += Pd[0 : G - 1]  # Q_m = P_{m+1} - 2 P_m + P_{m-1}
    R = Qm.reshape(16, 8, O_DIM, I_DIM).sum(axis=1)  # [16, O, I]
    Sp = np.cumsum(R, axis=0) - R  # S'_q = sum_{q'<q} R_{q'}
    L = Qm.reshape(16, 8, O_DIM, I_DIM).copy()
    L[:, 0] += Sp + Pd[0]
    blocks = np.concatenate(
        [
            L.reshape(G, O_DIM, I_DIM),
            (Qm.sum(axis=0) + Pd[0])[None],  # OV
            (Pd[0] - Pd[1])[None],  # RL
        ],
        axis=0,
    ).astype(np.float32)  # [130, O, I]
    qt = np.ascontiguousarray(blocks.transpose(2, 0, 1).reshape(I_DIM, NBLK * O_DIM))

    nb = np.zeros((128, 131), dtype=np.float32)
    nb[:, 0:128] = -np.arange(128, dtype=np.float32)[None, :]
    nb[:, 128] = -64.0
    nb[:, 129] = -63.5
    nb[:, 130] = Pd[0].sum(axis=1).astype(np.float32)  # C[o] = sum_i P_0[o,i]

    in_maps = []
    for c in range(N_CORES):
        xs = np.ascontiguousarray(x[:, c * BS : (c + 1) * BS])
        m = {"x": xs, "qt": qt, "nb": nb}
        if host_tables:
            # exact fallback: bucketize host-side with the provided tables
            exp_na = np.exp(-np.abs(xs))
            cdf = np.where(xs > 0, 1.0 - 0.5 * exp_na, 0.5 * exp_na).astype(np.float32)
            idx = np.clip((cdf * G).astype(np.int32), 0, G - 1)
            m["hg"] = idx.astype(np.float32)
            m["hbor"] = borders[idx].astype(np.float32)
            m["hil"] = inv_len[idx].astype(np.float32)
        in_maps.append(m)
    return in_maps, host_tables


_RUNNER_CACHE = {}


def _get_runner(host_tables, reps: int = 1):
    """Cached jitted 8-core runner (mirrors bass2jax.run_bass_via_pjrt multi-core path)."""
    key = (bool(host_tables), reps)
    if key in _RUNNER_CACHE:
        return _RUNNER_CACHE[key]

    import jax
    from jax.sharding import Mesh, PartitionSpec
    from jax.experimental.shard_map import shard_map
    import concourse.mybir as mybir
    from concourse.bass2jax import (
        _bass_exec_p,
        install_neuronx_cc_hook,
        partition_id_tensor,
    )

    install_neuronx_cc_hook()
    nc = _get_nc(host_tables, reps)
    if not nc.is_finalized():
        nc.finalize()
    assert nc.dbg_addr is None
    partition_name = nc.partition_id_tensor.name if nc.partition_id_tensor else None

    in_names, out_names, out_avals, zero_outs = [], [], [], []
    for alloc in nc.m.functions[0].allocations:
        if not isinstance(alloc, mybir.MemoryLocationSet):
            continue
        name = alloc.memorylocations[0].name
        if alloc.kind == "ExternalInput":
            if name != partition_name:
                in_names.append(name)
        elif alloc.kind == "ExternalOutput":
            shape = tuple(alloc.tensor_shape)
            dtype = mybir.dt.np(alloc.dtype)
            out_names.append(name)
            out_avals.append(jax.core.ShapedArray(shape, dtype))
            zero_outs.append(np.zeros(shape, dtype))
    n_params = len(in_names)
    all_names = in_names + out_names
    if partition_name is not None:
        all_names = all_names + [partition_name]

    def _body(*args):
        operands = list(args)
        if partition_name is not None:
            operands.append(partition_id_tensor())
        outs = _bass_exec_p.bind(
            *operands,
            out_avals=tuple(out_avals),
            in_names=tuple(all_names),
            out_names=tuple(out_names),
            lowering_input_output_aliases=(),
            sim_require_finite=True,
            sim_require_nnan=True,
            nc=nc,
        )
        return tuple(outs)

    devices = jax.devices()[:N_CORES]
    mesh = Mesh(np.asarray(devices), ("core",))
    n_outs = len(out_names)
    sharded = jax.jit(
        shard_map(
            _body,
            mesh=mesh,
            in_specs=(PartitionSpec("core"),) * (n_params + n_outs),
            out_specs=(PartitionSpec("core"),) * n_outs,
            check_rep=False,
        ),
        keep_unused=True,
    )

    def run(in_maps):
        concat_in = [
            np.concatenate([np.asarray(m[name]) for m in in_maps], axis=0)
            for name in in_names
        ]
        concat_zero = [
            np.zeros((N_CORES * z.shape[0], *z.shape[1:]), z.dtype) for z in zero_outs
        ]
        out_arrs = sharded(*concat_in, *concat_zero)
        res = [
            {
                name: np.asarray(out_arrs[i]).reshape(N_CORES, *out_avals[i].shape)[c]
                for i, name in enumerate(out_names)
            }
            for c in range(N_CORES)
        ]
        return res, (sharded, concat_in, concat_zero)

    _RUNNER_CACHE[key] = run
    return run


def _run(in_maps, host_tables, trace=False):
    run = _get_runner(host_tables)
    results, _ = run(in_maps)
    out = np.concatenate([r["out"] for r in results], axis=1)
    return np.ascontiguousarray(out.astype(np.float32)), results


def bench(in_maps, host_tables, iters=30, reps=1):
    """Return (best_per_exec_seconds, times list) by timing repeated dispatches."""
    import time
    import jax

    run = _get_runner(host_tables, reps)
    _, (sharded, concat_in, concat_zero) = run(in_maps)
    # device-resident inputs to avoid re-transfer
    din = [jax.device_put(a) for a in concat_in]
    dzero = [jax.device_put(a) for a in concat_zero]
    jax.block_until_ready(sharded(*din, *dzero))
    times = []
    for _ in range(iters):
        t0 = time.perf_counter()
        jax.block_until_ready(sharded(*din, *dzero))
        times.append(time.perf_counter() - t0)
    return min(times), times


def bench_device(in_maps, host_tables, reps=256, iters=10):
    """Estimate true per-kernel device time from the marginal cost between two
    loop counts of the SAME kernel structure.  The two dispatches are timed
    interleaved so slow drift in the (dominant, ~100ms) axon dispatch
    overhead cancels instead of biasing the difference."""
    import time
    import jax

    lo = max(2, reps // 8)
    runs = {}
    for r in (lo, reps):
        run = _get_runner(host_tables, r)
        _, (sharded, concat_in, concat_zero) = run(in_maps)
        din = [jax.device_put(a) for a in concat_in]
        dzero = [jax.device_put(a) for a in concat_zero]
        jax.block_until_ready(sharded(*din, *dzero))
        runs[r] = (sharded, din, dzero)
    t = {lo: [], reps: []}
    for _ in range(max(iters, 20)):
        for r in (lo, reps):
            sharded, din, dzero = runs[r]
            t0 = time.perf_counter()
            jax.block_until_ready(sharded(*din, *dzero))
            t[r].append(time.perf_counter() - t0)
    # dispatch overhead drifts between rounds but is common-mode within a
    # round, so the median of same-round differences is a robust estimate of
    # the marginal per-body device time
    diffs = sorted(b - a for a, b in zip(t[lo], t[reps]))
    per = diffs[len(diffs) // 2] / (reps - lo)
    return per, min(t[lo]), min(t[reps])


def kernel(x, func_parameter, borders, inverse_chunk_lengths):
    in_maps, host_tables = _host_inputs(x, func_parameter, borders, inverse_chunk_lengths)
    out, _ = _run(in_maps, host_tables, trace=False)
    return out


def kernel_with_stats(x, func_parameter, borders, inverse_chunk_lengths, trace=True):
    """Returns (out, results) - test harness helper."""
    in_maps, host_tables = _host_inputs(x, func_parameter, borders, inverse_chunk_lengths)
    out, results = _run(in_maps, host_tables)
    return out, (in_maps, host_tables)
